# revision 17
# baseline (speedup 1.0000x reference)
"""Trainium2 Bass kernel for nn_ColorHistograms.

Pipeline (per NeuronCore, 2 batch elements each, 8 cores):
  1. Stream x tiles [128 frames, 3888] from HBM, two 1MB DMAs per tile, ALL
     on the sync HWDGE ring (the scalar ring belongs to ScalarE's sequencer
     and the gpsimd ring carries the latency-critical staging/gather DMAs).
     Per-tile spatial sums: channel 0 on ScalarE (activation Copy +
     accum_out, raw), channels 1,2 in one strided VectorE reduce (raw).
  2. Per staging round (tiles 0-4, then 5-7): PE-transpose the raw sum
     columns to PSUM; one ScalarE activation applies scale=1/S (mean) and
     bias=-0.5 (centering) and casts to fp16; stage to a DRAM scratch in
     NREP replicas (mean-of-1296-uniforms is 0.5 +- 0.01 and only
     differences matter downstream, so fp16 keeps ~1e-3 relative accuracy
     while halving gather traffic). Round 0 covers tiles 0-4 because the
     first-half gather window reads means up to t=561.
  3. Toeplitz gather DMAs (w-chunks reading distinct replicas so the runtime
     assigns distinct SDMA engines) materialize the 101 shifted copies of
     the padded mean rows for each t-half, on the gpsimd SWDGE ring whose
     Q7 sequencer is otherwise idle (+ sync/scalar rings for batch 1's tail
     half, once the stream FIFO has drained).
  4. Base-row broadcast via a rank-1 PE matmul (ones[1,101]^T @ sh[0:1]) into
     PSUM; VectorE subtracts it from sh in place, |.| channel-reduce to fp32
     window features [101, 512] (+ ones row = bias trick). Only tiles 0 and
     7 need the out-of-range mask multiply.
  5. PE matmul [102]x[128 t]x[128 out] with fc weights (bias folded in as an
     extra contraction row), VectorE relu PSUM->SBUF, contiguous DMA out.

Every engine's instruction stream is chained with explicit ordering deps
(no semaphores) so the Tile scheduler's cost-model cannot reorder an
engine's queue away from the hand-tuned emission order: a misplaced
latency-bound op in an in-order queue stalls the engine and backpressures
the whole stream.
"""

import sys

if "/opt/trn_rl_repo" not in sys.path:
    sys.path.insert(0, "/opt/trn_rl_repo")

import numpy as np

N_CORES = 8
B, T, H, W, C = 16, 1024, 27, 48, 3
S = H * W                 # 1296 spatial positions
ROW = S * C               # 3888 floats per frame
LW = 101                  # lookup window
PAD = 50
OD = 128                  # output dim
BPC = B // N_CORES        # batches per core = 2
PADROW = T + LW - 1       # 1124
FT = T // 128             # 8 frame-tiles per batch
HFT = FT // 2             # 4 tiles per t-half
TH = T // 2               # 512, the t-half used to pipeline the tail
XCH = 2                   # chunk-DMAs per x tile
CENTER = 0.5              # mean centering applied in the fp16 staging cast
# The runtime picks the DMA engine from the DRAM-side address granule, so a
# gather whose reads all land in one ~13 KB region serializes onto a single
# engine (~24 GB/s). Stage the mean rows into NREP replicas spaced REPS
# elements apart and split each gather into w-chunks reading distinct
# replicas so the chunks land on distinct engines. The stride is an odd
# multiple of 2/4/8 KB so replicas stay distinct mod 16 for any granule size.
NREP = 4
REPS = 70656              # fp16 elements between replicas (141312 B)
# window rows are PERMUTED so that dest row 0 is w=50 (the base row): the
# rank-1 PE broadcast can then read partition 0 of the gather tile. fc
# weights and the mask are row-permuted on the host to match, so the matmul
# contraction is unchanged.
PERM = [50] + [w for w in range(LW) if w != 50]
# (dest_row0, dest_row1, src_w0, replica)
WCHUNKS = [(0, 1, 50, 3), (1, 26, 0, 0), (26, 51, 25, 1),
           (51, 76, 51, 2), (76, 101, 76, 3)]

_CACHE = {}


def _build_program():
    import concourse.bass as bass
    import concourse.tile as tile
    from concourse import bacc, mybir
    from concourse.ap import AP
    from concourse.tile_rust import add_dep_helper

    f32 = mybir.dt.float32
    f16 = mybir.dt.float16
    bf16 = mybir.dt.bfloat16
    nc = bacc.Bacc("TRN2", target_bir_lowering=False, debug=False)

    xs = nc.dram_tensor("xs", [BPC * T, ROW], f32, kind="ExternalInput")
    fcwb = nc.dram_tensor("fcwb", [LW + 1, OD], f32, kind="ExternalInput")
    maskw = nc.dram_tensor("maskw", [LW, T], bf16, kind="ExternalInput")
    ident = nc.dram_tensor("ident", [128, 128], f32, kind="ExternalInput")
    y = nc.dram_tensor("y", [BPC * T, OD], f32, kind="ExternalOutput")
    mcpad = nc.dram_tensor("mcpad", [NREP * REPS], f16)
    mc_ap = mcpad[:]

    def mc_view(offset, dims):
        return AP(tensor=mc_ap.tensor, offset=offset, ap=tuple(dims))

    # per-engine emission-order chains
    _last = {}

    def O(key, bi):
        ins = getattr(bi, "ins", bi)
        prev = _last.get(key)
        if prev is not None:
            # add_dep_helper(a, b): a depends on (waits for) b
            add_dep_helper(
                ins, prev, sync=False, reason="forced emission order"
            )
        _last[key] = ins
        return bi

    with tile.TileContext(nc) as tc:
        with (
            tc.tile_pool(name="consts", bufs=1) as consts,
            tc.tile_pool(name="xin", bufs=9) as xin,
            tc.tile_pool(name="junk", bufs=1) as junkp,
            tc.tile_pool(name="sums", bufs=2) as sumsp,
            tc.tile_pool(name="stg", bufs=2) as stgp,
            tc.tile_pool(name="gath", bufs=2) as gathp,
            tc.tile_pool(name="wf", bufs=2) as wfp,
            tc.tile_pool(name="outs", bufs=4) as outsp,
            tc.tile_pool(name="zrow", bufs=1) as zrowp,
            tc.tile_pool(name="pst", bufs=1, space="PSUM") as pst,
            tc.tile_pool(name="pba", bufs=1, space="PSUM") as pbap,
            tc.tile_pool(name="pso", bufs=4, space="PSUM") as pso,
        ):
            ones_sb = consts.tile([1, LW], f16)
            O("q7", nc.gpsimd.memset(ones_sb[:], 1.0))
            z = zrowp.tile([24, PADROW], f16)
            O("q7", nc.gpsimd.memset(z[:], 0.0))
            wf_t = {}
            for b in range(BPC):
                wf = wfp.tile([LW + 1, T], f32, tag="wf")
                wf_t[b] = wf
                # row LW must be 1.0 (bias trick); engines can only start at
                # partition 0/32/64/96: rows 96..100 are overwritten by the
                # window reduces
                O("q7", nc.gpsimd.memset(wf[96 : LW + 1, :], 1.0))
            fcwb_sb = consts.tile([LW + 1, OD], f32)
            O("q7", nc.gpsimd.dma_start(fcwb_sb[:], fcwb[:]))
            maskw_sb = consts.tile([LW, T], bf16)
            O("q7", nc.gpsimd.dma_start(maskw_sb[:], maskw[:]))
            ident_sb = consts.tile([128, 128], f32)
            O("q7", nc.gpsimd.dma_start(ident_sb[:], ident[:]))
            # zero-fill the used region of each replica (the padded wings
            # must read as 0.0; the inter-replica gaps are never read)
            O("q7", nc.gpsimd.dma_start(
                mc_view(0, [(REPS, NREP), (1, BPC * C * PADROW)]), z[:]
            ))

            # a tiny prewarm read spins up the SDMA engines before the
            # first 1MB x chunk's descriptors arrive
            warm = junkp.tile([128, 32], f32, tag="warm")
            O("sync", nc.sync.dma_start(warm[:], xs[0:128, 0:32]))

            # the full x stream on the sync HWDGE ring; emitted first so
            # the ring FIFO is pure-stream and in order
            xts = []
            for b in range(BPC):
                for i in range(FT):
                    xt = xin.tile([128, ROW], f32)
                    xts.append(xt)
                    for k in range(XCH):
                        lo, hi = k * (ROW // XCH), (k + 1) * (ROW // XCH)
                        O("sync", nc.sync.dma_start(
                            xt[:, lo:hi],
                            xs[b * T + i * 128 : b * T + (i + 1) * 128,
                               lo:hi],
                        ))

            # -------- emission helpers (order = per-engine priority) --------
            sums_t, sh_t, ba_t = {}, {}, {}

            def stageA_tile(b, i):
                # channel 0 on ScalarE (activation accum, raw; ScalarE must
                # stay light so its sequencer keeps pace), channels 1,2 in
                # one strided VectorE reduce (raw)
                sums = sums_t[b]
                xv = xts[b * FT + i][:].rearrange("p (s c) -> p c s", c=C)
                junk = junkp.tile([128, S], f32)
                O("act", nc.scalar.activation(
                    junk[:],
                    xv[:, 0, :],
                    mybir.ActivationFunctionType.Copy,
                    bias=0.0,
                    scale=1.0,
                    accum_out=sums[:, i * C : i * C + 1],
                ))
                O("v", nc.vector.tensor_reduce(
                    sums[:, i * C + 1 : i * C + 3],
                    xv[:, 1:3, :],
                    axis=mybir.AxisListType.X,
                    op=mybir.AluOpType.add,
                ))

            def stageA_tile_split(b, i):
                # the last tile is reduced per x-chunk so its means are
                # ready ~one chunk-DMA earlier; both engines' partials are
                # combined in one tiny VectorE add
                sums = sums_t[b]
                xt = xts[b * FT + i]
                stile = junkp.tile([128, 2 * C], f32, tag="stile")
                junk = junkp.tile([128, S], f32)
                for k in range(2):
                    lo, hi = k * (ROW // 2), (k + 1) * (ROW // 2)
                    xv = xt[:, lo:hi].rearrange("p (s c) -> p c s", c=C)
                    O("act", nc.scalar.activation(
                        junk[:, 0 : (hi - lo) // C],
                        xv[:, 0, :],
                        mybir.ActivationFunctionType.Copy,
                        bias=0.0,
                        scale=1.0,
                        accum_out=stile[:, k * C : k * C + 1],
                    ))
                    O("v", nc.vector.tensor_reduce(
                        stile[:, k * C + 1 : k * C + 3],
                        xv[:, 1:3, :],
                        axis=mybir.AxisListType.X,
                        op=mybir.AluOpType.add,
                    ))
                O("v", nc.vector.tensor_add(
                    sums[:, i * C : (i + 1) * C],
                    stile[:, 0:C],
                    stile[:, C : 2 * C],
                ))

            def stageA_finish(b, h):
                # transpose this round's raw sum columns to PSUM on PE, then
                # one ScalarE activation does mean-scale + center + fp16
                # cast, then stage to the DRAM replicas. Round 0 covers
                # tiles 0-4 (the h0 gather window reads means up to t=561);
                # round 1 covers 5-7.
                i0, i1 = (0, HFT + 1) if h == 0 else (HFT + 1, FT)
                nt = i1 - i0
                sums = sums_t[b]
                ps = pst.tile([C * nt, 128], f32, tag="pst")
                O("pe", nc.tensor.transpose(
                    ps[:], sums[:, i0 * C : i1 * C], ident_sb[:]
                ))
                stg = stgp.tile([C * nt, 128], f16, tag="stg")
                O("act", nc.scalar.activation(
                    stg[:],
                    ps[:],
                    mybir.ActivationFunctionType.Copy,
                    bias=-CENTER,
                    scale=1.0 / S,
                ))
                # the staging write is fp16->fp16 (the cast happened in the
                # activation), so HWDGE rings can carry it too; the tail
                # round spreads replicas across rings to parallelize the
                # descriptor generation
                if (b, h) == (1, 1):
                    stg_engs = [("q7", nc.gpsimd), ("sync", nc.sync),
                                ("act", nc.scalar), ("q7", nc.gpsimd)]
                else:
                    stg_engs = [("q7", nc.gpsimd)] * NREP
                for r in range(NREP):
                    name, eng = stg_engs[r]
                    O(name, eng.dma_start(
                        mc_view(
                            r * REPS + b * C * PADROW + PAD + i0 * 128,
                            [(128, nt), (PADROW, C), (1, 128)],
                        ),
                        stg[:],
                    ))

            def stageB_gather(b, h, engs):
                if h == 0:
                    sh = gathp.tile([LW, C * T], f16, tag="sh")
                    sh_t[b] = sh
                sh = sh_t[b]
                shd = sh[:].rearrange("p (c t) -> p c t", c=C)
                for ci, (r0, r1, w0, rep) in enumerate(WCHUNKS):
                    name, eng = engs[ci % len(engs)]
                    O(name, eng.dma_start(
                        shd[r0:r1, :, h * TH : (h + 1) * TH],
                        mc_view(
                            rep * REPS + b * C * PADROW + w0 + h * TH,
                            [(1, r1 - r0), (PADROW, C), (1, TH)],
                        ),
                    ))

            def stageBA(b, h):
                # rank-1 PE broadcast of the base row (permuted row 0) into
                # PSUM
                sh = sh_t[b]
                shv3 = sh[:].rearrange("p (c t) -> p c t", c=C)
                ba = pbap.tile([LW, C * TH], f32)
                ba_t[(b, h)] = ba
                for c in range(C):
                    O("pe", nc.tensor.matmul(
                        ba[:, c * TH : (c + 1) * TH],
                        ones_sb[:],
                        shv3[0:1, c, h * TH : (h + 1) * TH],
                    ))

            def stageDistC(b, h):
                # subtract the broadcast base row in place, |.|-reduce over
                # c, mask only the edge tiles, then matmul+relu+store the
                # half's 4 t-tiles
                sh, wf = sh_t[b], wf_t[b]
                shv3 = sh[:].rearrange("p (c t) -> p c t", c=C)
                ba = ba_t[(b, h)]
                bav = ba[:].rearrange("p (c t) -> p c t", c=C)
                O("v", nc.vector.tensor_sub(
                    shv3[:, :, h * TH : (h + 1) * TH],
                    shv3[:, :, h * TH : (h + 1) * TH],
                    bav,
                ))
                shv_tc = sh[:].rearrange("p (c t) -> p t c", c=C)
                O("v", nc.vector.tensor_reduce(
                    wf[0:LW, h * TH : (h + 1) * TH],
                    shv_tc[:, h * TH : (h + 1) * TH, :],
                    axis=mybir.AxisListType.X,
                    op=mybir.AluOpType.add,
                    apply_absolute_value=True,
                ))
                # out-of-range wings only exist for t<50 and t>=974
                if h == 0:
                    O("v", nc.vector.tensor_mul(
                        wf[0:LW, 0:128], wf[0:LW, 0:128], maskw_sb[:, 0:128]
                    ))
                else:
                    O("v", nc.vector.tensor_mul(
                        wf[0:LW, T - 128 : T],
                        wf[0:LW, T - 128 : T],
                        maskw_sb[:, T - 128 : T],
                    ))
                tail = b == 1 and h == 1
                for j in range(h * HFT, (h + 1) * HFT):
                    po = pso.tile([128, OD], f32)
                    O("pe", nc.tensor.matmul(
                        po[:], wf[:, bass.ts(j, 128)], fcwb_sb[:]
                    ))
                    osb = outsp.tile([128, OD], f32)
                    # the tail half's relus alternate DVE/ACT so the two
                    # final output pairs drain in parallel
                    if tail and j % 2 == 1:
                        O("act", nc.scalar.activation(
                            osb[:], po[:],
                            mybir.ActivationFunctionType.Relu,
                        ))
                    else:
                        O("v", nc.vector.tensor_scalar_max(
                            osb[:], po[:], 0.0
                        ))
                    # mid-stream outputs go on the gpsimd ring; the tail
                    # half spreads all four queues' descriptor generation
                    if not tail:
                        name, eng = "q7", nc.gpsimd
                    else:
                        name, eng = [
                            ("sync", nc.sync), ("act", nc.scalar),
                            ("q7", nc.gpsimd), ("q7", nc.gpsimd),
                        ][j - HFT]
                    O(name, eng.dma_start(
                        y[b * T + j * 128 : b * T + (j + 1) * 128, :], osb[:]
                    ))

            # -------- emission schedule --------
            # hand-ordered so every engine's (forced) in-order queue matches
            # the real readiness sequence; only batch 1's second half sits
            # in the post-stream tail.
            for b in range(BPC):
                sums_t[b] = sumsp.tile(
                    [128, C * FT], f32, name="sums", tag="sums"
                )
            for i in range(0, HFT + 1):
                stageA_tile(0, i)
            stageA_finish(0, 0)
            stageB_gather(0, 0, [("q7", nc.gpsimd)])
            stageBA(0, 0)
            for i in range(HFT + 1, FT):
                stageA_tile(0, i)
            stageA_finish(0, 1)
            stageB_gather(0, 1, [("q7", nc.gpsimd)])
            stageBA(0, 1)
            for i in range(0, 2):
                stageA_tile(1, i)
            stageDistC(0, 0)
            for i in range(2, 4):
                stageA_tile(1, i)
            stageDistC(0, 1)
            stageA_tile(1, 4)
            stageA_finish(1, 0)
            stageB_gather(1, 0, [("q7", nc.gpsimd)])
            stageBA(1, 0)
            for i in range(HFT + 1, FT - 1):
                stageA_tile(1, i)
            stageA_tile_split(1, FT - 1)
            stageA_finish(1, 1)
            stageB_gather(
                1, 1,
                [("q7", nc.gpsimd), ("sync", nc.sync), ("act", nc.scalar)],
            )
            stageDistC(1, 0)
            stageBA(1, 1)
            stageDistC(1, 1)

    nc.compile()
    return nc


def get_nc():
    if "nc" not in _CACHE:
        _CACHE["nc"] = _build_program()
    return _CACHE["nc"]


def make_host_inputs(x, fc_w, fc_b):
    """Per-core input maps from the full problem inputs."""
    x = np.ascontiguousarray(x, dtype=np.float32).reshape(B, T, ROW)
    wT = fc_w.T.astype(np.float32)[PERM]          # window-row permutation
    fcwb = np.concatenate([wT, fc_b[None, :].astype(np.float32)], axis=0)
    fcwb = np.ascontiguousarray(fcwb)
    u = np.arange(T)[None, :] + np.arange(LW)[:, None] - PAD
    import ml_dtypes

    maskw = ((u >= 0) & (u < T)).astype(ml_dtypes.bfloat16)[PERM]
    maskw = np.ascontiguousarray(maskw)
    ident = np.eye(128, dtype=np.float32)
    in_maps = []
    for ci in range(N_CORES):
        shard = np.ascontiguousarray(
            x[ci * BPC : (ci + 1) * BPC].reshape(BPC * T, ROW)
        )
        in_maps.append(
            {"xs": shard, "fcwb": fcwb, "maskw": maskw, "ident": ident}
        )
    return in_maps


def kernel(x, fc_w, fc_b):
    from concourse.bass_utils import run_bass_kernel_spmd

    nc = get_nc()
    in_maps = make_host_inputs(x, fc_w, fc_b)
    res = run_bass_kernel_spmd(nc, in_maps, list(range(N_CORES)))
    outs = [r["y"].reshape(BPC, T, OD) for r in res.results]
    return np.concatenate(outs, axis=0).astype(np.float32)


# revision 21
# speedup vs baseline: 1.0435x; 1.0435x over previous
"""Trainium2 Bass kernel for nn_ColorHistograms.

Pipeline (per NeuronCore, 2 batch elements each, 8 cores):
  1. Stream x tiles [128 frames, 3888] from HBM, two 1MB DMAs per tile, ALL
     on the sync HWDGE ring (the scalar ring belongs to ScalarE's sequencer
     and the gpsimd ring carries the latency-critical staging/gather DMAs).
     Per-tile spatial sums: channel 0 on ScalarE (activation Copy +
     accum_out, raw), channels 1,2 in one strided VectorE reduce (raw).
  2. Per staging round (tiles 0-4, then 5-7): PE-transpose the raw sum
     columns to PSUM; one ScalarE activation applies scale=1/S (mean) and
     bias=-0.5 (centering) and casts to fp16; stage to a DRAM scratch in
     NREP replicas (mean-of-1296-uniforms is 0.5 +- 0.01 and only
     differences matter downstream, so fp16 keeps ~1e-3 relative accuracy
     while halving gather traffic). Round 0 covers tiles 0-4 because the
     first-half gather window reads means up to t=561.
  3. Toeplitz gather DMAs (w-chunks reading distinct replicas so the runtime
     assigns distinct SDMA engines) materialize the 101 shifted copies of
     the padded mean rows for each t-half, on the gpsimd SWDGE ring whose
     Q7 sequencer is otherwise idle (+ sync/scalar rings for batch 1's tail
     half, once the stream FIFO has drained).
  4. Base-row broadcast via a rank-1 PE matmul (ones[1,101]^T @ sh[0:1]) into
     PSUM; VectorE subtracts it from sh in place, |.| channel-reduce to fp32
     window features [101, 512] (+ ones row = bias trick). Only tiles 0 and
     7 need the out-of-range mask multiply.
  5. PE matmul [102]x[128 t]x[128 out] with fc weights (bias folded in as an
     extra contraction row), VectorE relu PSUM->SBUF, contiguous DMA out.

Every engine's instruction stream is chained with explicit ordering deps
(no semaphores) so the Tile scheduler's cost-model cannot reorder an
engine's queue away from the hand-tuned emission order: a misplaced
latency-bound op in an in-order queue stalls the engine and backpressures
the whole stream.
"""

import sys

if "/opt/trn_rl_repo" not in sys.path:
    sys.path.insert(0, "/opt/trn_rl_repo")

import numpy as np

N_CORES = 8
B, T, H, W, C = 16, 1024, 27, 48, 3
S = H * W                 # 1296 spatial positions
ROW = S * C               # 3888 floats per frame
LW = 101                  # lookup window
PAD = 50
OD = 128                  # output dim
BPC = B // N_CORES        # batches per core = 2
PADROW = T + LW - 1       # 1124
FT = T // 128             # 8 frame-tiles per batch
HFT = FT // 2             # 4 tiles per t-half
TH = T // 2               # 512, the t-half used to pipeline the tail
XCH = 2                   # chunk-DMAs per x tile
CENTER = 0.5              # mean centering applied in the fp16 staging cast
# The runtime picks the DMA engine from the DRAM-side address granule, so a
# gather whose reads all land in one ~13 KB region serializes onto a single
# engine (~24 GB/s). Stage the mean rows into NREP replicas spaced REPS
# elements apart and split each gather into w-chunks reading distinct
# replicas so the chunks land on distinct engines. The stride is an odd
# multiple of 2/4/8 KB so replicas stay distinct mod 16 for any granule size.
NREP = 4
REPS = 70656              # fp16 elements between replicas (141312 B)
# window rows are PERMUTED so that dest row 0 is w=50 (the base row): the
# rank-1 PE broadcast can then read partition 0 of the gather tile. fc
# weights and the mask are row-permuted on the host to match, so the matmul
# contraction is unchanged.
PERM = [50] + [w for w in range(LW) if w != 50]
# (dest_row0, dest_row1, src_w0, replica)
WCHUNKS = [(0, 1, 50, 3), (1, 26, 0, 0), (26, 51, 25, 1),
           (51, 76, 51, 2), (76, 101, 76, 3)]

_CACHE = {}


def _build_program():
    import concourse.bass as bass
    import concourse.tile as tile
    from concourse import bacc, mybir
    from concourse.ap import AP
    from concourse.tile_rust import add_dep_helper

    f32 = mybir.dt.float32
    f16 = mybir.dt.float16
    bf16 = mybir.dt.bfloat16
    nc = bacc.Bacc("TRN2", target_bir_lowering=False, debug=False)

    xs = nc.dram_tensor("xs", [BPC * T, ROW], f32, kind="ExternalInput")
    fcwb = nc.dram_tensor("fcwb", [LW + 1, OD], f32, kind="ExternalInput")
    maskw = nc.dram_tensor("maskw", [LW, T], bf16, kind="ExternalInput")
    ident = nc.dram_tensor("ident", [128, 128], f32, kind="ExternalInput")
    y = nc.dram_tensor("y", [BPC * T, OD], f32, kind="ExternalOutput")
    mcpad = nc.dram_tensor("mcpad", [NREP * REPS], f16)
    mc_ap = mcpad[:]

    def mc_view(offset, dims):
        return AP(tensor=mc_ap.tensor, offset=offset, ap=tuple(dims))

    # per-engine emission-order chains
    _last = {}

    def O(key, bi):
        ins = getattr(bi, "ins", bi)
        prev = _last.get(key)
        if prev is not None:
            # add_dep_helper(a, b): a depends on (waits for) b
            add_dep_helper(
                ins, prev, sync=False, reason="forced emission order"
            )
        _last[key] = ins
        return bi

    with tile.TileContext(nc) as tc:
        with (
            tc.tile_pool(name="consts", bufs=1) as consts,
            tc.tile_pool(name="xin", bufs=9) as xin,
            tc.tile_pool(name="junk", bufs=1) as junkp,
            tc.tile_pool(name="sums", bufs=2) as sumsp,
            tc.tile_pool(name="stg", bufs=2) as stgp,
            tc.tile_pool(name="gath", bufs=2) as gathp,
            tc.tile_pool(name="wf", bufs=2) as wfp,
            tc.tile_pool(name="outs", bufs=4) as outsp,
            tc.tile_pool(name="zrow", bufs=1) as zrowp,
            tc.tile_pool(name="pst", bufs=1, space="PSUM") as pst,
            tc.tile_pool(name="pba", bufs=1, space="PSUM") as pbap,
            tc.tile_pool(name="pso", bufs=4, space="PSUM") as pso,
        ):
            ones_sb = consts.tile([1, LW], f16)
            O("q7", nc.gpsimd.memset(ones_sb[:], 1.0))
            z = zrowp.tile([24, PADROW], f16)
            O("q7", nc.gpsimd.memset(z[:], 0.0))
            wf_t = {}
            for b in range(BPC):
                wf = wfp.tile([LW + 1, T], f32, tag="wf")
                wf_t[b] = wf
                # row LW must be 1.0 (bias trick); engines can only start at
                # partition 0/32/64/96: rows 96..100 are overwritten by the
                # window reduces
                O("q7", nc.gpsimd.memset(wf[96 : LW + 1, :], 1.0))
            fcwb_sb = consts.tile([LW + 1, OD], f32)
            O("q7", nc.gpsimd.dma_start(fcwb_sb[:], fcwb[:]))
            maskw_sb = consts.tile([LW, T], bf16)
            O("q7", nc.gpsimd.dma_start(maskw_sb[:], maskw[:]))
            ident_sb = consts.tile([128, 128], f32)
            O("q7", nc.gpsimd.dma_start(ident_sb[:], ident[:]))
            # zero-fill the used region of each replica (the padded wings
            # must read as 0.0; the inter-replica gaps are never read)
            O("q7", nc.gpsimd.dma_start(
                mc_view(0, [(REPS, NREP), (1, BPC * C * PADROW)]), z[:]
            ))

            # the full x stream on the sync HWDGE ring; emitted first so
            # the ring FIFO is pure-stream and in order. The host ships x
            # CHANNEL-PLANAR ([c][s] per frame), so each chunk boundary is a
            # channel boundary: chunk 0 carries ch0+ch1, chunk 1 carries
            # ch2, and every reduce reads contiguous memory and can start
            # as soon as its own chunk lands.
            xts = []
            for b in range(BPC):
                for i in range(FT):
                    xt = xin.tile([128, ROW], f32)
                    xts.append(xt)
                    for lo, hi in ((0, 2 * S), (2 * S, ROW)):
                        O("sync", nc.sync.dma_start(
                            xt[:, lo:hi],
                            xs[b * T + i * 128 : b * T + (i + 1) * 128,
                               lo:hi],
                        ))

            # -------- emission helpers (order = per-engine priority) --------
            sums_t, sh_t, ba_t = {}, {}, {}

            def stageA_tile(b, i):
                # channel 0 on ScalarE (activation accum, raw; ScalarE must
                # stay light so its sequencer keeps pace), channels 1,2 as
                # separate contiguous VectorE reduces so ch1 starts on
                # chunk 0 and ch2 right when chunk 1 lands
                sums = sums_t[b]
                xt = xts[b * FT + i]
                junk = junkp.tile([128, S], f32)
                O("act", nc.scalar.activation(
                    junk[:],
                    xt[:, 0:S],
                    mybir.ActivationFunctionType.Copy,
                    bias=0.0,
                    scale=1.0,
                    accum_out=sums[:, i * C : i * C + 1],
                ))
                for ch in (1, 2):
                    O("v", nc.vector.tensor_reduce(
                        sums[:, i * C + ch : i * C + ch + 1],
                        xt[:, ch * S : (ch + 1) * S],
                        axis=mybir.AxisListType.X,
                        op=mybir.AluOpType.add,
                    ))

            def stageA_finish(b, h):
                # transpose this round's raw sum columns to PSUM on PE, then
                # one ScalarE activation does mean-scale + center + fp16
                # cast, then stage to the DRAM replicas. Round 0 covers
                # tiles 0-4 (the h0 gather window reads means up to t=561);
                # round 1 covers 5-7.
                i0, i1 = (0, HFT + 1) if h == 0 else (HFT + 1, FT)
                nt = i1 - i0
                sums = sums_t[b]
                ps = pst.tile([C * nt, 128], f32, tag="pst")
                O("pe", nc.tensor.transpose(
                    ps[:], sums[:, i0 * C : i1 * C], ident_sb[:]
                ))
                stg = stgp.tile([C * nt, 128], f16, tag="stg")
                O("act", nc.scalar.activation(
                    stg[:],
                    ps[:],
                    mybir.ActivationFunctionType.Copy,
                    bias=-CENTER,
                    scale=1.0 / S,
                ))
                for r in range(NREP):
                    O("q7", nc.gpsimd.dma_start(
                        mc_view(
                            r * REPS + b * C * PADROW + PAD + i0 * 128,
                            [(128, nt), (PADROW, C), (1, 128)],
                        ),
                        stg[:],
                    ))

            def stageB_gather(b, h, engs):
                if h == 0:
                    sh = gathp.tile([LW, C * T], f16, tag="sh")
                    sh_t[b] = sh
                sh = sh_t[b]
                shd = sh[:].rearrange("p (c t) -> p c t", c=C)
                for ci, (r0, r1, w0, rep) in enumerate(WCHUNKS):
                    name, eng = engs[ci % len(engs)]
                    O(name, eng.dma_start(
                        shd[r0:r1, :, h * TH : (h + 1) * TH],
                        mc_view(
                            rep * REPS + b * C * PADROW + w0 + h * TH,
                            [(1, r1 - r0), (PADROW, C), (1, TH)],
                        ),
                    ))

            def stageBA(b, h):
                # rank-1 PE broadcast of the base row (permuted row 0) into
                # PSUM
                sh = sh_t[b]
                shv3 = sh[:].rearrange("p (c t) -> p c t", c=C)
                ba = pbap.tile([LW, C * TH], f32)
                ba_t[(b, h)] = ba
                for c in range(C):
                    O("pe", nc.tensor.matmul(
                        ba[:, c * TH : (c + 1) * TH],
                        ones_sb[:],
                        shv3[0:1, c, h * TH : (h + 1) * TH],
                    ))

            def stageDistC(b, h):
                # subtract the broadcast base row in place, |.|-reduce over
                # c, mask only the edge tiles, then matmul+relu+store the
                # half's 4 t-tiles
                sh, wf = sh_t[b], wf_t[b]
                shv3 = sh[:].rearrange("p (c t) -> p c t", c=C)
                ba = ba_t[(b, h)]
                bav = ba[:].rearrange("p (c t) -> p c t", c=C)
                O("v", nc.vector.tensor_sub(
                    shv3[:, :, h * TH : (h + 1) * TH],
                    shv3[:, :, h * TH : (h + 1) * TH],
                    bav,
                ))
                shv_tc = sh[:].rearrange("p (c t) -> p t c", c=C)
                O("v", nc.vector.tensor_reduce(
                    wf[0:LW, h * TH : (h + 1) * TH],
                    shv_tc[:, h * TH : (h + 1) * TH, :],
                    axis=mybir.AxisListType.X,
                    op=mybir.AluOpType.add,
                    apply_absolute_value=True,
                ))
                # out-of-range wings only exist for t<50 and t>=974
                if h == 0:
                    O("v", nc.vector.tensor_mul(
                        wf[0:LW, 0:128], wf[0:LW, 0:128], maskw_sb[:, 0:128]
                    ))
                else:
                    O("v", nc.vector.tensor_mul(
                        wf[0:LW, T - 128 : T],
                        wf[0:LW, T - 128 : T],
                        maskw_sb[:, T - 128 : T],
                    ))
                for j in range(h * HFT, (h + 1) * HFT):
                    po = pso.tile([128, OD], f32)
                    O("pe", nc.tensor.matmul(
                        po[:], wf[:, bass.ts(j, 128)], fcwb_sb[:]
                    ))
                    osb = outsp.tile([128, OD], f32)
                    O("v", nc.vector.tensor_scalar_max(osb[:], po[:], 0.0))
                    # mid-stream outputs go on the gpsimd ring; batch 1's
                    # tail half splits across sync+scalar (idle and drained
                    # by then, and off the gpsimd ring carrying gathers)
                    if b == 0 or h == 0:
                        name, eng = "q7", nc.gpsimd
                    else:
                        name, eng = (
                            ("sync", nc.sync) if j % 2 == 0
                            else ("act", nc.scalar)
                        )
                    O(name, eng.dma_start(
                        y[b * T + j * 128 : b * T + (j + 1) * 128, :], osb[:]
                    ))

            # -------- emission schedule --------
            # hand-ordered so every engine's (forced) in-order queue matches
            # the real readiness sequence; only batch 1's second half sits
            # in the post-stream tail.
            for b in range(BPC):
                sums_t[b] = sumsp.tile(
                    [128, C * FT], f32, name="sums", tag="sums"
                )
            for i in range(0, HFT + 1):
                stageA_tile(0, i)
            stageA_finish(0, 0)
            stageB_gather(0, 0, [("q7", nc.gpsimd)])
            stageBA(0, 0)
            for i in range(HFT + 1, FT):
                stageA_tile(0, i)
            stageA_finish(0, 1)
            stageB_gather(0, 1, [("q7", nc.gpsimd)])
            stageBA(0, 1)
            for i in range(0, 2):
                stageA_tile(1, i)
            stageDistC(0, 0)
            for i in range(2, 4):
                stageA_tile(1, i)
            stageDistC(0, 1)
            stageA_tile(1, 4)
            stageA_finish(1, 0)
            stageB_gather(1, 0, [("q7", nc.gpsimd)])
            stageBA(1, 0)
            for i in range(HFT + 1, FT):
                stageA_tile(1, i)
            stageA_finish(1, 1)
            stageB_gather(
                1, 1,
                [("q7", nc.gpsimd), ("sync", nc.sync), ("act", nc.scalar)],
            )
            stageDistC(1, 0)
            stageBA(1, 1)
            stageDistC(1, 1)

    nc.compile()
    return nc


def get_nc():
    if "nc" not in _CACHE:
        _CACHE["nc"] = _build_program()
    return _CACHE["nc"]


def make_host_inputs(x, fc_w, fc_b):
    """Per-core input maps from the full problem inputs."""
    # ship x channel-planar ([c][s] per frame) so every on-device reduce
    # reads contiguous memory
    x = np.asarray(x, dtype=np.float32).reshape(B, T, S, C)
    x = np.ascontiguousarray(x.transpose(0, 1, 3, 2)).reshape(B, T, ROW)
    wT = fc_w.T.astype(np.float32)[PERM]          # window-row permutation
    fcwb = np.concatenate([wT, fc_b[None, :].astype(np.float32)], axis=0)
    fcwb = np.ascontiguousarray(fcwb)
    u = np.arange(T)[None, :] + np.arange(LW)[:, None] - PAD
    import ml_dtypes

    maskw = ((u >= 0) & (u < T)).astype(ml_dtypes.bfloat16)[PERM]
    maskw = np.ascontiguousarray(maskw)
    ident = np.eye(128, dtype=np.float32)
    in_maps = []
    for ci in range(N_CORES):
        shard = np.ascontiguousarray(
            x[ci * BPC : (ci + 1) * BPC].reshape(BPC * T, ROW)
        )
        in_maps.append(
            {"xs": shard, "fcwb": fcwb, "maskw": maskw, "ident": ident}
        )
    return in_maps


def kernel(x, fc_w, fc_b):
    from concourse.bass_utils import run_bass_kernel_spmd

    nc = get_nc()
    in_maps = make_host_inputs(x, fc_w, fc_b)
    res = run_bass_kernel_spmd(nc, in_maps, list(range(N_CORES)))
    outs = [r["y"].reshape(BPC, T, OD) for r in res.results]
    return np.concatenate(outs, axis=0).astype(np.float32)
